# revision 63
# baseline (speedup 1.0000x reference)
"""Trainium2 Bass kernel for nn_LlamaAttention_cam (sparse attention + CaM merge).

Sharding: tensor-parallel over heads across 8 NeuronCores (2 heads/core).
Each core computes its heads' QKV projections, RoPE, masked attention
(start+recent keep mask), CaM rank-1 correction for the last chunk, and a
partial o_proj (its 256 columns of x against the matching 256 rows of Wo^T).
The host sums the 8 partial outputs (the reduction of the head-parallel
o_proj), which replaces the all-reduce.

Optimizations over the straightforward implementation:
- bf16 matmul operands everywhere (fp32 PSUM accumulation); ~22 MB of
  DMA per core instead of ~89 MB.
- one-shot weight/hidden-state loads from host-prearranged 2D layouts,
  spread across the SP/Activation/GpSimd DMA queues; Wq is head-major and
  split so head 0's Q-projection starts after a quarter of the load.
- K/V projections and RoPE pruned to the kept keys ([0,204) u [1229,2048));
  the keep-mask is folded into the exp bias (no separate mask multiply).
- CaM: strictly-recent sum via an all-ones-except-row-77 reduction vector,
  single-element cross-partition fetch of E[1229, 2047], and the rank-1
  merge matmuls deferred into the o_proj phase so their serial scalar
  chain stays off the PE critical path.
- o_proj: l-outer loop (stationary operand reused by 4 consecutive
  matmuls), batched per-t-chunk writeback alternating across three DMA
  queues; the last chunk drains in 512-column pieces.
"""

import sys

for _p in ("/opt/trn_rl_repo",):
    if _p not in sys.path:
        sys.path.append(_p)

import numpy as np

import concourse.bass as bass
import concourse.mybir as mybir
import concourse.tile as tile
from concourse import bacc, bass_utils

F32 = mybir.dt.float32
BF = mybir.dt.bfloat16
AF = mybir.ActivationFunctionType

T = 2048
DM = 2048
H = 16
D = 128
NCORES = 8
HL = H // NCORES          # heads per core = 2
JC = HL * D               # local attn width = 256
SB = 204                  # start keep
RB = 819                  # recent keep
EV = T - RB               # 1229 (first recent key; CaM source row)
KC = DM // 128            # 16 model-dim chunks
TB = T // 512             # 4 t-blocks of 512
TI = T // 128             # 16 t-chunks of 128
# kept key blocks: (block idx, kept row range within block)
KBLK = [(0, 0, 128), (1, 0, 76), (9, 77, 128)] + [(b, 0, 128) for b in range(10, 16)]
NB = len(KBLK)            # 9
# K-projection kept column ranges per t-block (in-tile lo, hi)
KKEEP = {0: (0, SB), 2: (EV - 1024, 512), 3: (0, 512)}
# V t-chunks that hold kept keys (blocks 0,1 and 9..15)
VCHUNKS = [0, 1, 9, 10, 11, 12, 13, 14, 15]


def _build_nc(reps=1):
    nc = bacc.Bacc("TRN2", target_bir_lowering=False, debug=False,
                   num_devices=NCORES)
    # host-prearranged layouts: hsr[p, (tb kc t)], w*[p, (kc j)]
    hsr = nc.dram_tensor("hsr", [128, TB * KC * 512], BF,
                         kind="ExternalInput").ap()
    wqT = nc.dram_tensor("wqT", [128, KC * JC], BF, kind="ExternalInput").ap()
    wkT = nc.dram_tensor("wkT", [128, KC * JC], BF, kind="ExternalInput").ap()
    wvT = nc.dram_tensor("wvT", [128, KC * JC], BF, kind="ExternalInput").ap()
    woT = nc.dram_tensor("woT", [JC, DM], BF, kind="ExternalInput").ap()
    cosT = nc.dram_tensor("cosT", [D, T], BF, kind="ExternalInput").ap()
    sinTs = nc.dram_tensor("sinTs", [D, T], BF, kind="ExternalInput").ap()
    u2 = nc.dram_tensor("u2", [1, HL], F32, kind="ExternalInput").ap()
    masks = nc.dram_tensor("masks", [128, 2], F32, kind="ExternalInput").ap()
    # all-ones except row 77 (key 1229): reduces E block 9 to the strictly
    # recent keys 1230.. in one matmul
    o9b = nc.dram_tensor("o9b", [128, 1], BF, kind="ExternalInput").ap()
    po = nc.dram_tensor("po", [T, DM], BF, kind="ExternalOutput").ap()
    dbg = nc.dram_tensor("dbg", [1, 16], F32, kind="ExternalOutput").ap()

    with tile.TileContext(nc) as tc:
      for _rep in range(reps):
        with (
            tc.tile_pool(name="resid", bufs=1) as pres,        # long-lived
            tc.tile_pool(name="hst", bufs=4) as phst,          # [128, 16*512] bf16
            tc.tile_pool(name="E", bufs=16) as pE,             # [128,512] bf16
            tc.tile_pool(name="ropetr", bufs=8) as ptr,        # [128,512] bf16
            tc.tile_pool(name="tmpf", bufs=4) as ptmp,         # [128,512] f32
            tc.tile_pool(name="osb", bufs=3) as posb,          # [128,2048] bf16
            tc.tile_pool(name="rows", bufs=4) as prow,         # small [1,*]
            tc.tile_pool(name="ps", bufs=6, space="PSUM") as pps,
            tc.tile_pool(name="psdn", bufs=1, space="PSUM") as ppsd,
        ):
            # ---- one-shot loads, ordered so the Q-proj of t-block 0 can
            # start as early as possible ----
            # spread the startup loads across engine DMA queues so their
            # fixed issue overheads overlap; transfers share the DMA fabric
            # wq is head-major [p, (l kc j)] so head 0's Q-proj only needs
            # the first half of the load
            wq_sb = pres.tile([128, KC * JC], BF, tag="wq")
            nc.sync.dma_start(wq_sb[:, 0:KC * 128], wqT[:, 0:KC * 128])
            hstt = [phst.tile([128, KC * 512], BF, tag="hst", name=f"hst{tb}")
                    for tb in range(TB - 1)]
            nc.sync.dma_start(hstt[0][:, 0:4 * 512], hsr[:, 0:4 * 512])
            nc.sync.dma_start(wq_sb[:, KC * 128:KC * JC],
                              wqT[:, KC * 128:KC * JC])
            nc.sync.dma_start(hstt[0][:, 4 * 512:16 * 512],
                              hsr[:, 4 * 512:16 * 512])
            wk_sb = pres.tile([128, KC * JC], BF, tag="wk")
            nc.scalar.dma_start(wk_sb[:], wkT[:])
            cos_sb = pres.tile([D, T], BF, tag="cos")
            sin_sb = pres.tile([D, T], BF, tag="sin")
            nc.gpsimd.dma_start(cos_sb[:], cosT[:])
            nc.gpsimd.dma_start(sin_sb[:], sinTs[:])
            nc.scalar.dma_start(hstt[1][:],
                                hsr[:, 1 * KC * 512:2 * KC * 512])
            u2_sb = pres.tile([1, HL], F32, tag="u2")
            nc.gpsimd.dma_start(u2_sb[:], u2[:])
            masks_sb = pres.tile([128, 2], F32, tag="masks")
            nc.gpsimd.dma_start(masks_sb[:], masks[:])
            mask1 = masks_sb[:, 0:1]   # exp bias: 0 where kept, -30 evicted
            mask9 = masks_sb[:, 1:2]
            o9b_sb = pres.tile([128, 1], BF, tag="o9b")
            nc.gpsimd.dma_start(o9b_sb[:], o9b[:])
            wv_sb = pres.tile([128, KC * JC], BF, tag="wv")
            nc.sync.dma_start(wv_sb[:], wvT[:])
            nc.gpsimd.dma_start(hstt[2][:],
                                hsr[:, 2 * KC * 512:3 * KC * 512])
            hst3 = phst.tile([128, KC * 512], BF, tag="hst", name="hst3")
            nc.gpsimd.dma_start(hst3[:],
                                hsr[:, 3 * KC * 512:4 * KC * 512])
            wo_sb = [pres.tile([128, DM], BF, tag=f"wo{l}", name=f"wo{l}")
                     for l in range(HL)]
            for l in range(HL):
                nc.sync.dma_start(wo_sb[l][:], woT[l * 128:(l + 1) * 128, :])

            ones = pres.tile([128, 1], BF, tag="ones")
            nc.vector.memset(ones[:], 1.0)

            # rope'd q/k in [d, t] bf16; v in [t(keys), d_local] bf16
            qrT = [pres.tile([D, T], BF, tag=f"qrT{l}", name=f"qrT{l}")
                   for l in range(HL)]
            krT = [pres.tile([D, T], BF, tag=f"krT{l}", name=f"krT{l}")
                   for l in range(HL)]
            vt = pres.tile([128, TI * JC], BF, tag="vt")
            outT = [pres.tile([D, T], BF, tag=f"outT{l}", name=f"outT{l}")
                    for l in range(HL)]

            # zero the evicted slivers of krT read by the partial key blocks
            for l in range(HL):
                nc.vector.memset(krT[l][:, SB:256], 0.0)
                nc.vector.memset(krT[l][:, 1152:EV], 0.0)

            def rope(ps_in, dst, c0, w):
                """dst[:, :w](bf16) = rope(ps_in[:, :w]) at positions c0..c0+w."""
                raw = ptr.tile([128, 512], BF, tag="tr")
                nc.scalar.copy(raw[:, 0:w], ps_in[:, 0:w])
                sh = ptr.tile([128, 512], BF, tag="tr")
                nc.sync.dma_start(sh[0:64, 0:w], raw[64:128, 0:w])
                nc.sync.dma_start(sh[64:128, 0:w], raw[0:64, 0:w])
                t1 = ptr.tile([128, 512], BF, tag="tr")
                nc.vector.tensor_mul(t1[:, 0:w], raw[:, 0:w], cos_sb[:, c0:c0 + w])
                t2 = ptr.tile([128, 512], BF, tag="tr")
                nc.vector.tensor_mul(t2[:, 0:w], sh[:, 0:w], sin_sb[:, c0:c0 + w])
                nc.vector.tensor_add(dst, t1[:, 0:w], t2[:, 0:w])

            # ---------------- phase 1+2: projections + rope ----------------
            hstt.append(hst3)
            for tb in range(TB):
                t0 = tb * 512
                hst = hstt[tb]
                for l in range(HL):
                    # Q projection (all positions)
                    psq = pps.tile([128, 512], F32, tag="ps")
                    for kc in range(KC):
                        nc.tensor.matmul(
                            psq[:], wq_sb[:, (l * KC + kc) * 128:
                                          (l * KC + kc) * 128 + 128],
                            hst[:, kc * 512:(kc + 1) * 512],
                            start=(kc == 0), stop=(kc == KC - 1))
                    rope(psq, qrT[l][:, t0:t0 + 512], t0, 512)
                    # K projection (kept positions only)
                    if tb in KKEEP:
                        lo, hi = KKEEP[tb]
                        w = hi - lo
                        psk = pps.tile([128, 512], F32, tag="ps")
                        for kc in range(KC):
                            nc.tensor.matmul(
                                psk[:, 0:w],
                                wk_sb[:, kc * JC + l * 128: kc * JC + l * 128 + 128],
                                hst[:, kc * 512 + lo: kc * 512 + hi],
                                start=(kc == 0), stop=(kc == KC - 1))
                        rope(psk, krT[l][:, t0 + lo:t0 + hi], t0 + lo, w)
                # V projection for this t-block's kept chunks
                for ti in VCHUNKS:
                    if ti // 4 != tb:
                        continue
                    j = ti % 4
                    psv = pps.tile([128, JC], F32, tag="ps")
                    for kc in range(KC):
                        nc.tensor.matmul(
                            psv[:], hst[:, kc * 512 + j * 128: kc * 512 + j * 128 + 128],
                            wv_sb[:, kc * JC:(kc + 1) * JC],
                            start=(kc == 0), stop=(kc == KC - 1))
                    nc.scalar.copy(vt[:, ti * JC:(ti + 1) * JC], psv[:])

            # ---------------- phase 3: attention per head / t-block ----------------
            rbf3 = [pres.tile([128, 512], F32, tag=f"rbf3{l}", name=f"rbf3{l}")
                    for l in range(HL)]
            cam = {}   # l -> (coef, vrow) for the deferred rank-1 merge
            for l in range(HL):
                for tb in range(TB):
                    ts5 = slice(tb * 512, tb * 512 + 512)
                    E = []
                    erow_bf = None
                    for (b, r0, r1) in KBLK:
                        pst = pps.tile([128, 512], F32, tag="ps")
                        nc.tensor.matmul(pst[:],
                                         krT[l][:, b * 128:(b + 1) * 128],
                                         qrT[l][:, ts5], start=True, stop=True)
                        e = pE.tile([128, 512], BF, tag="E")
                        if r0 != 0 or r1 != 128:
                            m = mask1 if b == 1 else mask9
                            nc.scalar.activation(e[:], pst[:], AF.Exp, bias=m)
                        else:
                            nc.scalar.activation(e[:], pst[:], AF.Exp)
                        E.append(e)
                        if tb == TB - 1 and b == 9:
                            # E value of key 1229 at the last query t=2047
                            erow_bf = prow.tile([1, 1], BF, tag="sc_b")
                            nc.gpsimd.dma_start(erow_bf[:], e[77:78, 511:512])
                    psav = pps.tile([128, 512], F32, tag="ps")
                    psdn = ppsd.tile([1, 512], F32, tag="dn2")
                    for bi, (b, r0, r1) in enumerate(KBLK):
                        nc.tensor.matmul(psav[:],
                                         vt[:, b * JC + l * D: b * JC + (l + 1) * D],
                                         E[bi][:], start=(bi == 0),
                                         stop=(bi == NB - 1))
                        nc.tensor.matmul(psdn[:], ones[:], E[bi][:],
                                         start=(bi == 0), stop=(bi == NB - 1))
                    dn_sb = prow.tile([1, 512], F32, tag="row512")
                    nc.vector.tensor_copy(dn_sb[:], psdn[:])
                    recip = prow.tile([1, 512], F32, tag="row512")
                    nc.vector.reciprocal(recip[:], dn_sb[:])

                    if tb == TB - 1:
                        # ---- CaM: bernoulli draw; rank-1 merge is deferred ----
                        # srec = sum over strictly-recent keys 1230..
                        # (o9b zeroes row 77 = key 1229; bias mask zeroed <77)
                        pssr = ppsd.tile([1, 256], F32, tag="dn")
                        for bi, (b, r0, r1) in enumerate(KBLK[2:]):
                            o = o9b_sb if b == 9 else ones
                            nc.tensor.matmul(pssr[:], o[:],
                                             E[2 + bi][:, 256:512],
                                             start=(bi == 0),
                                             stop=(bi == NB - 3))
                        srec = prow.tile([1, 256], F32, tag="row256")
                        nc.vector.tensor_copy(srec[:], pssr[:])
                        erow = prow.tile([1, 1], F32, tag="sc")
                        nc.vector.tensor_copy(erow[:], erow_bf[:])
                        # scalars at t = 2047
                        r_last = recip[0:1, 511:512]
                        num = prow.tile([1, 1], F32, tag="sc")
                        nc.vector.tensor_mul(num[:], erow[:], r_last)
                        mean = prow.tile([1, 1], F32, tag="sc")
                        nc.vector.tensor_mul(mean[:], srec[0:1, 255:256], r_last)
                        nc.vector.tensor_scalar_mul(mean[:], mean[:], 1.0 / 818.0)
                        nc.vector.tensor_scalar_add(mean[:], mean[:], 1e-6)
                        um = prow.tile([1, 1], F32, tag="sc")
                        nc.vector.tensor_mul(um[:], u2_sb[0:1, l:l + 1], mean[:])
                        bern = prow.tile([1, 1], F32, tag="sc")
                        nc.vector.tensor_tensor(bern[:], um[:], num[:],
                                                mybir.AluOpType.is_lt)
                        bs = prow.tile([1, 1], F32, tag="sc")
                        nc.vector.tensor_scalar_mul(bs[:], bern[:], 1.0 / RB)
                        coef = prow.tile([1, 256], BF, tag="row256b")
                        nc.vector.tensor_scalar_mul(coef[:], srec[:], bs[:])
                        dbgrow = prow.tile([1, 8], F32, tag="dbgrow")
                        nc.vector.tensor_copy(dbgrow[0:1, 0:1], num[:])
                        nc.vector.tensor_copy(dbgrow[0:1, 1:2], mean[:])
                        nc.vector.tensor_copy(dbgrow[0:1, 2:3], bern[:])
                        nc.vector.tensor_copy(dbgrow[0:1, 3:4], srec[0:1, 255:256])
                        nc.vector.tensor_copy(dbgrow[0:1, 4:5], um[:])
                        nc.vector.tensor_copy(dbgrow[0:1, 5:6], u2_sb[0:1, l:l + 1])
                        nc.vector.tensor_copy(dbgrow[0:1, 6:7], r_last)
                        nc.vector.tensor_copy(dbgrow[0:1, 7:8], erow[:])
                        nc.sync.dma_start(dbg[0:1, l * 8:(l + 1) * 8], dbgrow[:])
                        vrow = prow.tile([1, D], BF, tag="vrow")
                        nc.gpsimd.dma_start(
                            vrow[:], vt[77:78, 9 * JC + l * D: 9 * JC + (l + 1) * D])
                        cam[l] = (coef, vrow)
                    # normalize columns by 1/denom, store bf16
                    rbf = (rbf3[l] if tb == TB - 1
                           else ptmp.tile([128, 512], F32, tag="tmp"))
                    nc.gpsimd.partition_broadcast(rbf[:], recip[:])
                    nc.vector.tensor_mul(outT[l][:, ts5], psav[:], rbf[:])

            # ---------------- phase 4: partial o_proj ----------------
            # t-chunks 14,15 read the CaM-corrected tail of outT, so the
            # deferred CaM merge is emitted between ti=13 and ti=14 — by then
            # its serial DVE chain has long finished.
            po_eng = [nc.sync, nc.scalar, nc.gpsimd]

            def oproj(ti, split_po=False):
                # l outer so 4 consecutive matmuls share one stationary
                # operand (outT t-chunk) — the weight load amortizes 4x
                osb = posb.tile([128, DM], BF, tag="osb")
                psos = [pps.tile([128, 512], F32, tag="ps",
                                 name=f"pso{ti}_{mb}") for mb in range(TB)]
                for l in range(HL):
                    for mb in range(TB):
                        nc.tensor.matmul(psos[mb][:],
                                         outT[l][:, ti * 128:(ti + 1) * 128],
                                         wo_sb[l][:, mb * 512:(mb + 1) * 512],
                                         start=(l == 0), stop=(l == HL - 1))
                for mb in range(TB):
                    ms = slice(mb * 512, (mb + 1) * 512)
                    if (ti * TB + mb) % 2 == 0:
                        nc.scalar.copy(osb[:, ms], psos[mb][:])
                    else:
                        nc.vector.tensor_copy(osb[:, ms], psos[mb][:])
                    if split_po:
                        # drain the tail in 512-col pieces as copies finish
                        po_eng[(ti + mb) % 3].dma_start(
                            po[ti * 128:(ti + 1) * 128, ms], osb[:, ms])
                if not split_po:
                    po_eng[ti % 3].dma_start(po[ti * 128:(ti + 1) * 128, :],
                                             osb[:])

            for ti in range(TI - 5):
                oproj(ti)

            # deferred CaM rank-1 merge (off the critical PE path)
            for l in range(HL):
                coef, vrow = cam[l]
                pscr = pps.tile([128, 256], F32, tag="ps")
                nc.tensor.matmul(pscr[:], vrow[:], coef[:], start=True, stop=True)
                corr = ptr.tile([128, 512], BF, tag="tr")
                nc.vector.tensor_mul(corr[:, 0:256], pscr[:], rbf3[l][:, 256:512])
                nc.vector.tensor_add(outT[l][:, 1792:2048],
                                     outT[l][:, 1792:2048], corr[:, 0:256])

            for ti in range(TI - 5, TI):
                oproj(ti, split_po=(ti == TI - 1))

    nc.compile()
    return nc


_NC_CACHE = None


def _get_nc():
    global _NC_CACHE
    if _NC_CACHE is None:
        _NC_CACHE = _build_nc()
    return _NC_CACHE


def make_in_maps(hidden_states, Wq, Wk, Wv, Wo):
    import ml_dtypes
    BF16 = ml_dtypes.bfloat16

    hs = np.asarray(hidden_states, np.float32).reshape(T, DM)
    hs = np.nan_to_num(hs, nan=0.0, posinf=1e4, neginf=-1e4)
    # hsr[p, (tb kc t)] = hs[tb*512+t, kc*128+p]
    hsr = np.ascontiguousarray(
        hs.T.astype(BF16).reshape(KC, 128, TB, 512)
        .transpose(1, 2, 0, 3).reshape(128, TB * KC * 512))

    def wlay(w):  # [DM, JC] -> [128, (kc j)]
        return np.ascontiguousarray(
            w.reshape(KC, 128, JC).transpose(1, 0, 2).reshape(128, KC * JC))

    def wlay_hm(w):  # [DM, JC] -> [128, (l kc j)] head-major
        return np.ascontiguousarray(
            w.reshape(KC, 128, HL, D).transpose(1, 2, 0, 3)
            .reshape(128, KC * JC))
    Wq = np.asarray(Wq, np.float32)
    Wk = np.asarray(Wk, np.float32)
    Wv = np.asarray(Wv, np.float32)
    Wo = np.asarray(Wo, np.float32)

    inv_freq = 1.0 / (10000.0 ** (np.arange(0, D, 2, dtype=np.float32) / D))
    freqs = np.arange(T, dtype=np.float32)[:, None] * inv_freq[None, :]
    emb = np.concatenate([freqs, freqs], axis=-1)          # [T, D]
    cosT = np.ascontiguousarray(np.cos(emb).T.astype(BF16))
    sinT = np.sin(emb).T.astype(np.float32)
    sinTs = np.ascontiguousarray(
        np.concatenate([-sinT[:D // 2], sinT[D // 2:]], axis=0).astype(BF16))

    import jax
    import jax.numpy as jnp
    u_full = np.asarray(
        jax.random.uniform(jax.random.key(42), (1, H), jnp.float32))

    # exp-bias masks: 0 where the key is kept, -30 where evicted
    # (exp(score - 30) ~ 1e-13 vs denominators > 1 -> negligible)
    mask_np = np.full((128, 2), -30.0, np.float32)
    mask_np[:76, 0] = 0.0
    mask_np[77:, 1] = 0.0
    o9b_np = np.ones((128, 1), BF16)
    o9b_np[77, 0] = 0.0

    scale = 1.0 / np.sqrt(np.float32(D))
    in_maps = []
    for c in range(NCORES):
        js = slice(c * JC, (c + 1) * JC)
        in_maps.append({
            "hsr": hsr,
            "wqT": wlay_hm((Wq[js, :].T * scale).astype(BF16)),
            "wkT": wlay(Wk[js, :].T.astype(BF16)),
            "wvT": wlay(Wv[js, :].T.astype(BF16)),
            "woT": np.ascontiguousarray(Wo[:, js].T.astype(BF16)),
            "cosT": cosT,
            "sinTs": sinTs,
            "u2": np.ascontiguousarray(u_full[:, c * HL:(c + 1) * HL]),
            "masks": mask_np,
            "o9b": o9b_np,
        })
    return in_maps


def kernel(hidden_states, Wq, Wk, Wv, Wo):
    nc = _get_nc()
    in_maps = make_in_maps(hidden_states, Wq, Wk, Wv, Wo)
    res = bass_utils.run_bass_kernel_spmd(nc, in_maps,
                                          core_ids=list(range(NCORES)))
    out = np.zeros((T, DM), np.float64)
    for c in range(NCORES):
        out += res.results[c]["po"].astype(np.float64)
    out = np.nan_to_num(out.astype(np.float32), nan=0.0, posinf=1e4,
                        neginf=-1e4)
    return out.reshape(1, T, DM)


# revision 64
# speedup vs baseline: 1.3238x; 1.3238x over previous
"""Trainium2 Bass kernel for nn_LlamaAttention_cam (sparse attention + CaM merge).

Sharding: tensor-parallel over heads across 8 NeuronCores (2 heads/core).
Each core computes its heads' QKV projections, RoPE, masked attention
(start+recent keep mask), CaM rank-1 correction for the last chunk, and a
partial o_proj (its 256 columns of x against the matching 256 rows of Wo^T).
The host sums the 8 partial outputs (the reduction of the head-parallel
o_proj), which replaces the all-reduce.

Optimizations over the straightforward implementation:
- bf16 matmul operands everywhere (fp32 PSUM accumulation); ~22 MB of
  DMA per core instead of ~89 MB.
- one-shot weight/hidden-state loads from host-prearranged 2D layouts,
  spread across the SP/Activation/GpSimd DMA queues; Wq is head-major and
  split so head 0's Q-projection starts after a quarter of the load.
- K/V projections and RoPE pruned to the kept keys ([0,204) u [1229,2048));
  the keep-mask is folded into the exp bias (no separate mask multiply).
- CaM: strictly-recent sum via an all-ones-except-row-77 reduction vector,
  single-element cross-partition fetch of E[1229, 2047], and the rank-1
  merge matmuls deferred into the o_proj phase so their serial scalar
  chain stays off the PE critical path.
- o_proj: l-outer loop (stationary operand reused by 4 consecutive
  matmuls), batched per-t-chunk writeback alternating across three DMA
  queues; the last chunk drains in 512-column pieces.
"""

import sys

for _p in ("/opt/trn_rl_repo",):
    if _p not in sys.path:
        sys.path.append(_p)

import numpy as np

import concourse.bass as bass
import concourse.mybir as mybir
import concourse.tile as tile
from concourse import bacc, bass_utils

F32 = mybir.dt.float32
BF = mybir.dt.bfloat16
AF = mybir.ActivationFunctionType

T = 2048
DM = 2048
H = 16
D = 128
NCORES = 8
HL = H // NCORES          # heads per core = 2
JC = HL * D               # local attn width = 256
SB = 204                  # start keep
RB = 819                  # recent keep
EV = T - RB               # 1229 (first recent key; CaM source row)
KC = DM // 128            # 16 model-dim chunks
TB = T // 512             # 4 t-blocks of 512
TI = T // 128             # 16 t-chunks of 128
# kept key blocks: (block idx, kept row range within block)
KBLK = [(0, 0, 128), (1, 0, 76), (9, 77, 128)] + [(b, 0, 128) for b in range(10, 16)]
NB = len(KBLK)            # 9
# K-projection kept column ranges per t-block (in-tile lo, hi)
KKEEP = {0: (0, SB), 2: (EV - 1024, 512), 3: (0, 512)}
# V t-chunks that hold kept keys (blocks 0,1 and 9..15)
VCHUNKS = [0, 1, 9, 10, 11, 12, 13, 14, 15]


def _build_nc(reps=1):
    nc = bacc.Bacc("TRN2", target_bir_lowering=False, debug=False,
                   num_devices=NCORES)
    # host-prearranged layouts: hsr[p, (tb kc t)], w*[p, (kc j)]
    hsr = nc.dram_tensor("hsr", [128, TB * KC * 512], BF,
                         kind="ExternalInput").ap()
    wqT = nc.dram_tensor("wqT", [128, KC * JC], BF, kind="ExternalInput").ap()
    wkT = nc.dram_tensor("wkT", [128, KC * JC], BF, kind="ExternalInput").ap()
    wvT = nc.dram_tensor("wvT", [128, KC * JC], BF, kind="ExternalInput").ap()
    woT = nc.dram_tensor("woT", [JC, DM], BF, kind="ExternalInput").ap()
    cosT = nc.dram_tensor("cosT", [D, T], BF, kind="ExternalInput").ap()
    sinTs = nc.dram_tensor("sinTs", [D, T], BF, kind="ExternalInput").ap()
    u2 = nc.dram_tensor("u2", [1, HL], F32, kind="ExternalInput").ap()
    masks = nc.dram_tensor("masks", [128, 2], F32, kind="ExternalInput").ap()
    # all-ones except row 77 (key 1229): reduces E block 9 to the strictly
    # recent keys 1230.. in one matmul
    o9b = nc.dram_tensor("o9b", [128, 1], BF, kind="ExternalInput").ap()
    po = nc.dram_tensor("po", [T, DM], BF, kind="ExternalOutput").ap()
    dbg = nc.dram_tensor("dbg", [1, 16], F32, kind="ExternalOutput").ap()

    with tile.TileContext(nc) as tc:
      for _rep in range(reps):
        with (
            tc.tile_pool(name="resid", bufs=1) as pres,        # long-lived
            tc.tile_pool(name="hst", bufs=4) as phst,          # [128, 16*512] bf16
            tc.tile_pool(name="E", bufs=16) as pE,             # [128,512] bf16
            tc.tile_pool(name="ropetr", bufs=8) as ptr,        # [128,512] bf16
            tc.tile_pool(name="tmpf", bufs=4) as ptmp,         # [128,512] f32
            tc.tile_pool(name="osb", bufs=3) as posb,          # [128,2048] bf16
            tc.tile_pool(name="rows", bufs=4) as prow,         # small [1,*]
            tc.tile_pool(name="ps", bufs=6, space="PSUM") as pps,
            tc.tile_pool(name="psdn", bufs=1, space="PSUM") as ppsd,
        ):
            # ---- one-shot loads, ordered so the Q-proj of t-block 0 can
            # start as early as possible ----
            # spread the startup loads across engine DMA queues so their
            # fixed issue overheads overlap; transfers share the DMA fabric
            # wq is head-major [p, (l kc j)] so head 0's Q-proj only needs
            # the first half of the load
            wq_sb = pres.tile([128, KC * JC], BF, tag="wq")
            nc.sync.dma_start(wq_sb[:, 0:KC * 128], wqT[:, 0:KC * 128])
            hstt = [phst.tile([128, KC * 512], BF, tag="hst", name=f"hst{tb}")
                    for tb in range(TB - 1)]
            # 4-chunk pieces so the first Q-projection chain can track the
            # arrivals instead of waiting for one big transfer
            for q in range(4):
                cs = slice(q * 4 * 512, (q + 1) * 4 * 512)
                nc.sync.dma_start(hstt[0][:, cs], hsr[:, cs])
            nc.sync.dma_start(wq_sb[:, KC * 128:KC * JC],
                              wqT[:, KC * 128:KC * JC])
            wk_sb = pres.tile([128, KC * JC], BF, tag="wk")
            nc.scalar.dma_start(wk_sb[:], wkT[:])
            cos_sb = pres.tile([D, T], BF, tag="cos")
            sin_sb = pres.tile([D, T], BF, tag="sin")
            nc.gpsimd.dma_start(cos_sb[:], cosT[:])
            nc.gpsimd.dma_start(sin_sb[:], sinTs[:])
            for q in range(2):
                cs = slice((2 + q) * 8 * 512, (3 + q) * 8 * 512)
                nc.scalar.dma_start(hstt[1][:, cs.start - KC * 512:
                                            cs.stop - KC * 512], hsr[:, cs])
            u2_sb = pres.tile([1, HL], F32, tag="u2")
            nc.gpsimd.dma_start(u2_sb[:], u2[:])
            masks_sb = pres.tile([128, 2], F32, tag="masks")
            nc.gpsimd.dma_start(masks_sb[:], masks[:])
            mask1 = masks_sb[:, 0:1]   # exp bias: 0 where kept, -30 evicted
            mask9 = masks_sb[:, 1:2]
            o9b_sb = pres.tile([128, 1], BF, tag="o9b")
            nc.gpsimd.dma_start(o9b_sb[:], o9b[:])
            wv_sb = pres.tile([128, KC * JC], BF, tag="wv")
            nc.sync.dma_start(wv_sb[:], wvT[:])
            nc.gpsimd.dma_start(hstt[2][:],
                                hsr[:, 2 * KC * 512:3 * KC * 512])
            hst3 = phst.tile([128, KC * 512], BF, tag="hst", name="hst3")
            nc.gpsimd.dma_start(hst3[:],
                                hsr[:, 3 * KC * 512:4 * KC * 512])
            wo_sb = [pres.tile([128, DM], BF, tag=f"wo{l}", name=f"wo{l}")
                     for l in range(HL)]
            for l in range(HL):
                nc.sync.dma_start(wo_sb[l][:], woT[l * 128:(l + 1) * 128, :])

            ones = pres.tile([128, 1], BF, tag="ones")
            nc.vector.memset(ones[:], 1.0)

            # rope'd q/k in [d, t] bf16; v in [t(keys), d_local] bf16
            qrT = [pres.tile([D, T], BF, tag=f"qrT{l}", name=f"qrT{l}")
                   for l in range(HL)]
            krT = [pres.tile([D, T], BF, tag=f"krT{l}", name=f"krT{l}")
                   for l in range(HL)]
            vt = pres.tile([128, TI * JC], BF, tag="vt")
            outT = [pres.tile([D, T], BF, tag=f"outT{l}", name=f"outT{l}")
                    for l in range(HL)]

            # zero the evicted slivers of krT read by the partial key blocks
            for l in range(HL):
                nc.vector.memset(krT[l][:, SB:256], 0.0)
                nc.vector.memset(krT[l][:, 1152:EV], 0.0)

            def rope(ps_in, dst, c0, w):
                """dst[:, :w](bf16) = rope(ps_in[:, :w]) at positions c0..c0+w."""
                raw = ptr.tile([128, 512], BF, tag="tr")
                nc.scalar.copy(raw[:, 0:w], ps_in[:, 0:w])
                sh = ptr.tile([128, 512], BF, tag="tr")
                nc.sync.dma_start(sh[0:64, 0:w], raw[64:128, 0:w])
                nc.sync.dma_start(sh[64:128, 0:w], raw[0:64, 0:w])
                t1 = ptr.tile([128, 512], BF, tag="tr")
                nc.vector.tensor_mul(t1[:, 0:w], raw[:, 0:w], cos_sb[:, c0:c0 + w])
                t2 = ptr.tile([128, 512], BF, tag="tr")
                nc.vector.tensor_mul(t2[:, 0:w], sh[:, 0:w], sin_sb[:, c0:c0 + w])
                nc.vector.tensor_add(dst, t1[:, 0:w], t2[:, 0:w])

            # ---------------- phase 1+2: projections + rope ----------------
            hstt.append(hst3)
            for tb in range(TB):
                t0 = tb * 512
                hst = hstt[tb]
                for l in range(HL):
                    # Q projection (all positions)
                    psq = pps.tile([128, 512], F32, tag="ps")
                    for kc in range(KC):
                        nc.tensor.matmul(
                            psq[:], wq_sb[:, (l * KC + kc) * 128:
                                          (l * KC + kc) * 128 + 128],
                            hst[:, kc * 512:(kc + 1) * 512],
                            start=(kc == 0), stop=(kc == KC - 1))
                    rope(psq, qrT[l][:, t0:t0 + 512], t0, 512)
                    # K projection (kept positions only)
                    if tb in KKEEP:
                        lo, hi = KKEEP[tb]
                        w = hi - lo
                        psk = pps.tile([128, 512], F32, tag="ps")
                        for kc in range(KC):
                            nc.tensor.matmul(
                                psk[:, 0:w],
                                wk_sb[:, kc * JC + l * 128: kc * JC + l * 128 + 128],
                                hst[:, kc * 512 + lo: kc * 512 + hi],
                                start=(kc == 0), stop=(kc == KC - 1))
                        rope(psk, krT[l][:, t0 + lo:t0 + hi], t0 + lo, w)
                # V projection for this t-block's kept chunks
                for ti in VCHUNKS:
                    if ti // 4 != tb:
                        continue
                    j = ti % 4
                    psv = pps.tile([128, JC], F32, tag="ps")
                    for kc in range(KC):
                        nc.tensor.matmul(
                            psv[:], hst[:, kc * 512 + j * 128: kc * 512 + j * 128 + 128],
                            wv_sb[:, kc * JC:(kc + 1) * JC],
                            start=(kc == 0), stop=(kc == KC - 1))
                    nc.scalar.copy(vt[:, ti * JC:(ti + 1) * JC], psv[:])

            # ---------------- phase 3: attention per head / t-block ----------------
            rbf3 = [pres.tile([128, 512], F32, tag=f"rbf3{l}", name=f"rbf3{l}")
                    for l in range(HL)]
            cam = {}   # l -> (coef, vrow) for the deferred rank-1 merge
            for l in range(HL):
                for tb in range(TB):
                    ts5 = slice(tb * 512, tb * 512 + 512)
                    E = []
                    erow_bf = None
                    for (b, r0, r1) in KBLK:
                        pst = pps.tile([128, 512], F32, tag="ps")
                        nc.tensor.matmul(pst[:],
                                         krT[l][:, b * 128:(b + 1) * 128],
                                         qrT[l][:, ts5], start=True, stop=True)
                        e = pE.tile([128, 512], BF, tag="E")
                        if r0 != 0 or r1 != 128:
                            m = mask1 if b == 1 else mask9
                            nc.scalar.activation(e[:], pst[:], AF.Exp, bias=m)
                        else:
                            nc.scalar.activation(e[:], pst[:], AF.Exp)
                        E.append(e)
                        if tb == TB - 1 and b == 9:
                            # E value of key 1229 at the last query t=2047
                            erow_bf = prow.tile([1, 1], BF, tag="sc_b")
                            nc.gpsimd.dma_start(erow_bf[:], e[77:78, 511:512])
                    psav = pps.tile([128, 512], F32, tag="ps")
                    psdn = ppsd.tile([1, 512], F32, tag="dn2")
                    for bi, (b, r0, r1) in enumerate(KBLK):
                        nc.tensor.matmul(psav[:],
                                         vt[:, b * JC + l * D: b * JC + (l + 1) * D],
                                         E[bi][:], start=(bi == 0),
                                         stop=(bi == NB - 1))
                        nc.tensor.matmul(psdn[:], ones[:], E[bi][:],
                                         start=(bi == 0), stop=(bi == NB - 1))
                    dn_sb = prow.tile([1, 512], F32, tag="row512")
                    nc.vector.tensor_copy(dn_sb[:], psdn[:])
                    recip = prow.tile([1, 512], F32, tag="row512")
                    nc.vector.reciprocal(recip[:], dn_sb[:])

                    if tb == TB - 1:
                        # ---- CaM: bernoulli draw; rank-1 merge is deferred ----
                        # srec = sum over strictly-recent keys 1230..
                        # (o9b zeroes row 77 = key 1229; bias mask zeroed <77)
                        pssr = ppsd.tile([1, 256], F32, tag="dn")
                        for bi, (b, r0, r1) in enumerate(KBLK[2:]):
                            o = o9b_sb if b == 9 else ones
                            nc.tensor.matmul(pssr[:], o[:],
                                             E[2 + bi][:, 256:512],
                                             start=(bi == 0),
                                             stop=(bi == NB - 3))
                        srec = prow.tile([1, 256], F32, tag="row256")
                        nc.vector.tensor_copy(srec[:], pssr[:])
                        erow = prow.tile([1, 1], F32, tag="sc")
                        nc.vector.tensor_copy(erow[:], erow_bf[:])
                        # scalars at t = 2047
                        r_last = recip[0:1, 511:512]
                        num = prow.tile([1, 1], F32, tag="sc")
                        nc.vector.tensor_mul(num[:], erow[:], r_last)
                        mean = prow.tile([1, 1], F32, tag="sc")
                        nc.vector.tensor_mul(mean[:], srec[0:1, 255:256], r_last)
                        nc.vector.tensor_scalar_mul(mean[:], mean[:], 1.0 / 818.0)
                        nc.vector.tensor_scalar_add(mean[:], mean[:], 1e-6)
                        um = prow.tile([1, 1], F32, tag="sc")
                        nc.vector.tensor_mul(um[:], u2_sb[0:1, l:l + 1], mean[:])
                        bern = prow.tile([1, 1], F32, tag="sc")
                        nc.vector.tensor_tensor(bern[:], um[:], num[:],
                                                mybir.AluOpType.is_lt)
                        bs = prow.tile([1, 1], F32, tag="sc")
                        nc.vector.tensor_scalar_mul(bs[:], bern[:], 1.0 / RB)
                        coef = prow.tile([1, 256], BF, tag="row256b")
                        nc.vector.tensor_scalar_mul(coef[:], srec[:], bs[:])
                        dbgrow = prow.tile([1, 8], F32, tag="dbgrow")
                        nc.vector.tensor_copy(dbgrow[0:1, 0:1], num[:])
                        nc.vector.tensor_copy(dbgrow[0:1, 1:2], mean[:])
                        nc.vector.tensor_copy(dbgrow[0:1, 2:3], bern[:])
                        nc.vector.tensor_copy(dbgrow[0:1, 3:4], srec[0:1, 255:256])
                        nc.vector.tensor_copy(dbgrow[0:1, 4:5], um[:])
                        nc.vector.tensor_copy(dbgrow[0:1, 5:6], u2_sb[0:1, l:l + 1])
                        nc.vector.tensor_copy(dbgrow[0:1, 6:7], r_last)
                        nc.vector.tensor_copy(dbgrow[0:1, 7:8], erow[:])
                        nc.sync.dma_start(dbg[0:1, l * 8:(l + 1) * 8], dbgrow[:])
                        vrow = prow.tile([1, D], BF, tag="vrow")
                        nc.gpsimd.dma_start(
                            vrow[:], vt[77:78, 9 * JC + l * D: 9 * JC + (l + 1) * D])
                        cam[l] = (coef, vrow)
                    # normalize columns by 1/denom, store bf16
                    rbf = (rbf3[l] if tb == TB - 1
                           else ptmp.tile([128, 512], F32, tag="tmp"))
                    nc.gpsimd.partition_broadcast(rbf[:], recip[:])
                    nc.vector.tensor_mul(outT[l][:, ts5], psav[:], rbf[:])

            # ---------------- phase 4: partial o_proj ----------------
            # t-chunks 14,15 read the CaM-corrected tail of outT, so the
            # deferred CaM merge is emitted between ti=13 and ti=14 — by then
            # its serial DVE chain has long finished.
            po_eng = [nc.sync, nc.scalar, nc.gpsimd]

            def oproj(ti, split_po=False):
                # l outer so 4 consecutive matmuls share one stationary
                # operand (outT t-chunk) — the weight load amortizes 4x
                osb = posb.tile([128, DM], BF, tag="osb")
                psos = [pps.tile([128, 512], F32, tag="ps",
                                 name=f"pso{ti}_{mb}") for mb in range(TB)]
                for l in range(HL):
                    for mb in range(TB):
                        nc.tensor.matmul(psos[mb][:],
                                         outT[l][:, ti * 128:(ti + 1) * 128],
                                         wo_sb[l][:, mb * 512:(mb + 1) * 512],
                                         start=(l == 0), stop=(l == HL - 1))
                for mb in range(TB):
                    ms = slice(mb * 512, (mb + 1) * 512)
                    if (ti * TB + mb) % 2 == 0:
                        nc.scalar.copy(osb[:, ms], psos[mb][:])
                    else:
                        nc.vector.tensor_copy(osb[:, ms], psos[mb][:])
                    if split_po:
                        # drain the tail in 512-col pieces as copies finish
                        po_eng[(ti + mb) % 3].dma_start(
                            po[ti * 128:(ti + 1) * 128, ms], osb[:, ms])
                if not split_po:
                    po_eng[ti % 3].dma_start(po[ti * 128:(ti + 1) * 128, :],
                                             osb[:])

            for ti in range(TI - 5):
                oproj(ti)

            # deferred CaM rank-1 merge (off the critical PE path)
            for l in range(HL):
                coef, vrow = cam[l]
                pscr = pps.tile([128, 256], F32, tag="ps")
                nc.tensor.matmul(pscr[:], vrow[:], coef[:], start=True, stop=True)
                corr = ptr.tile([128, 512], BF, tag="tr")
                nc.vector.tensor_mul(corr[:, 0:256], pscr[:], rbf3[l][:, 256:512])
                nc.vector.tensor_add(outT[l][:, 1792:2048],
                                     outT[l][:, 1792:2048], corr[:, 0:256])

            for ti in range(TI - 5, TI):
                oproj(ti, split_po=(ti == TI - 1))

    nc.compile()
    return nc


_NC_CACHE = None


def _get_nc():
    global _NC_CACHE
    if _NC_CACHE is None:
        _NC_CACHE = _build_nc()
    return _NC_CACHE


def make_in_maps(hidden_states, Wq, Wk, Wv, Wo):
    import ml_dtypes
    BF16 = ml_dtypes.bfloat16

    hs = np.asarray(hidden_states, np.float32).reshape(T, DM)
    hs = np.nan_to_num(hs, nan=0.0, posinf=1e4, neginf=-1e4)
    # hsr[p, (tb kc t)] = hs[tb*512+t, kc*128+p]
    hsr = np.ascontiguousarray(
        hs.T.astype(BF16).reshape(KC, 128, TB, 512)
        .transpose(1, 2, 0, 3).reshape(128, TB * KC * 512))

    def wlay(w):  # [DM, JC] -> [128, (kc j)]
        return np.ascontiguousarray(
            w.reshape(KC, 128, JC).transpose(1, 0, 2).reshape(128, KC * JC))

    def wlay_hm(w):  # [DM, JC] -> [128, (l kc j)] head-major
        return np.ascontiguousarray(
            w.reshape(KC, 128, HL, D).transpose(1, 2, 0, 3)
            .reshape(128, KC * JC))
    Wq = np.asarray(Wq, np.float32)
    Wk = np.asarray(Wk, np.float32)
    Wv = np.asarray(Wv, np.float32)
    Wo = np.asarray(Wo, np.float32)

    inv_freq = 1.0 / (10000.0 ** (np.arange(0, D, 2, dtype=np.float32) / D))
    freqs = np.arange(T, dtype=np.float32)[:, None] * inv_freq[None, :]
    emb = np.concatenate([freqs, freqs], axis=-1)          # [T, D]
    cosT = np.ascontiguousarray(np.cos(emb).T.astype(BF16))
    sinT = np.sin(emb).T.astype(np.float32)
    sinTs = np.ascontiguousarray(
        np.concatenate([-sinT[:D // 2], sinT[D // 2:]], axis=0).astype(BF16))

    import jax
    import jax.numpy as jnp
    u_full = np.asarray(
        jax.random.uniform(jax.random.key(42), (1, H), jnp.float32))

    # exp-bias masks: 0 where the key is kept, -30 where evicted
    # (exp(score - 30) ~ 1e-13 vs denominators > 1 -> negligible)
    mask_np = np.full((128, 2), -30.0, np.float32)
    mask_np[:76, 0] = 0.0
    mask_np[77:, 1] = 0.0
    o9b_np = np.ones((128, 1), BF16)
    o9b_np[77, 0] = 0.0

    scale = 1.0 / np.sqrt(np.float32(D))
    in_maps = []
    for c in range(NCORES):
        js = slice(c * JC, (c + 1) * JC)
        in_maps.append({
            "hsr": hsr,
            "wqT": wlay_hm((Wq[js, :].T * scale).astype(BF16)),
            "wkT": wlay(Wk[js, :].T.astype(BF16)),
            "wvT": wlay(Wv[js, :].T.astype(BF16)),
            "woT": np.ascontiguousarray(Wo[:, js].T.astype(BF16)),
            "cosT": cosT,
            "sinTs": sinTs,
            "u2": np.ascontiguousarray(u_full[:, c * HL:(c + 1) * HL]),
            "masks": mask_np,
            "o9b": o9b_np,
        })
    return in_maps


def kernel(hidden_states, Wq, Wk, Wv, Wo):
    nc = _get_nc()
    in_maps = make_in_maps(hidden_states, Wq, Wk, Wv, Wo)
    res = bass_utils.run_bass_kernel_spmd(nc, in_maps,
                                          core_ids=list(range(NCORES)))
    out = np.zeros((T, DM), np.float64)
    for c in range(NCORES):
        out += res.results[c]["po"].astype(np.float64)
    out = np.nan_to_num(out.astype(np.float32), nan=0.0, posinf=1e4,
                        neginf=-1e4)
    return out.reshape(1, T, DM)
